# revision 16
# baseline (speedup 1.0000x reference)
"""Trainium2 Bass kernel for: out = relu(L0@(X@W0) + L1@(X@W1) + L2@(X@W2) + bias).

Shapes: X [8192, 32], Lk [8192, 8192], Wk [32, 32], bias [32] (all f32).

Strategy (8 NeuronCores, node-dim sharding):
  - Each core owns a 1024-row block of the output: C_c = sum_g Lg[rows_c] @ (X @ Wg).
  - Computed transposed on-chip:  C_c.T = sum_g (X@Wg).T @ Lg[rows_c].T
    so the big Lg data is the *moving* PE operand (streamed through the array)
    and the tiny Y = X@Wg tiles are the stationary weights.
  - PE contracts over the partition dim, so Lg must be laid out with the
    contraction index (Lg's column index) on partitions.  f32 DMA-transpose
    doesn't exist on TRN2, so the transpose is done host-side as part of
    sharding: each core receives LgT_c = Lg[rows_c].T contiguous [8192, 1024].
    That makes every device-side DMA a contiguous line-rate stream - the
    kernel is purely HBM-bandwidth bound (96 MB/core), per the target regime.
  - float32r matmuls: single-pass PE streaming (fp32 needs 2 half-rate
    passes); walrus requires f32r operands to be produced as f32r, so L/X/W
    are declared f32r end-to-end (bit-identical 4-byte storage).
  - Phase A computes Y = X@Wg for all g in one batched pass: X.T is packed
    [128, n/4] (4 row-groups on partitions) and all three W's are multiplied
    concurrently via tile_position row-packing.
  - Per (m-chunk of 512 nodes): one PSUM bank accumulates all 3*64 = 192
    matmuls (graphs x k-tiles); epilogue = ScalarE Relu(acc + bias) -> SBUF
    -> DMA out.  Output per core is C_c.T [32, 1024]; host transposes back.
"""

import numpy as np

import concourse.bacc as bacc
import concourse.mybir as mybir
import concourse.tile as tile
from concourse.bass_utils import run_bass_kernel_spmd

N = 8192
C = 32
N_CORES = 8
ROWS = N // N_CORES  # 1024

P = 128          # SBUF partitions / PE contraction tile
R = 4            # X.T row-group packing factor
MM_N = 512       # max f32 moving free dim (one PSUM bank)
T_PACK = 4       # k-tiles per DMA (DMA size = T_PACK * ROWS * 512B)
LT_BUFS = 7      # L-tile prefetch depth
N_RINGS = 3      # DMA rings to rotate over: sync, scalar (HWDGE), gpsimd (SWDGE)
USE_F32R = True  # float32r matmul (accuracy checked in test.py)


def build_nc(n=N, rows=ROWS, c=C, t_pack=T_PACK, lt_bufs=LT_BUFS,
             use_f32r=USE_F32R, debug=False):
    f32 = mybir.dt.float32
    fmm = mybir.dt.float32r if use_f32r else f32
    kt_total = n // P            # k-tiles per graph
    ni = kt_total // t_pack      # DMA iterations per graph
    mc_cnt = (rows + MM_N - 1) // MM_N
    J = kt_total // R            # inner k-tile groups for phase A packing
    c3 = 3 * c

    nc = bacc.Bacc("TRN2", target_bir_lowering=False, debug=debug)

    XT4 = nc.dram_tensor("XT4", [P, n // R], fmm, kind="ExternalInput")
    Wc4 = nc.dram_tensor("Wcat4", [P, c3], fmm, kind="ExternalInput")
    B = nc.dram_tensor("bias", [c], f32, kind="ExternalInput")
    LT = [nc.dram_tensor(f"L{g}T", [n, rows], fmm, kind="ExternalInput")
          for g in range(3)]
    OUT = nc.dram_tensor("out", [c, rows], f32, kind="ExternalOutput")

    with tile.TileContext(nc) as tc:
        with (
            tc.tile_pool(name="const", bufs=1) as cpool,
            tc.tile_pool(name="ypool", bufs=1) as ypool,
            tc.tile_pool(name="lpool", bufs=lt_bufs) as lpool,
            tc.tile_pool(name="opool", bufs=1) as opool,
            tc.tile_pool(name="apsum", bufs=1, space="PSUM") as apsum,
            tc.tile_pool(name="mpsum", bufs=1, space="PSUM") as mpsum,
        ):
            xt4 = cpool.tile([P, n // R], fmm)
            wc4 = cpool.tile([P, c3], fmm)
            bs = cpool.tile([c, 1], f32)
            nc.sync.dma_start(xt4[:], XT4[:])
            nc.scalar.dma_start(wc4[:], Wc4[:])
            nc.scalar.dma_start(bs[:], B[:][:, None])

            # Phase A: Y_g = X @ Wg for all g at once.
            # ys col layout: (kt % J)*3c*R + (kt // J)*3c + g*c
            ys = ypool.tile([P, kt_total * c3], fmm)
            # Concurrent row-group MMs must land in distinct PSUM banks.
            for j in range(J):
                for s in range(R):
                    pa = apsum.tile([P, c3], f32, tag=f"pa{s}", name=f"pa{s}")
                    nc.tensor.matmul(
                        pa[:],
                        xt4[32 * s:32 * (s + 1), j * P:(j + 1) * P],
                        wc4[32 * s:32 * (s + 1), :],
                        start=True, stop=True,
                        tile_position=(32 * s, 0),
                    )
                    nc.vector.tensor_copy(
                        ys[:, (j * R + s) * c3:(j * R + s + 1) * c3], pa[:])

            # Main: acc_m[:c, :] += Y_g[ktile].T @ LgT[ktile, m-chunk]
            accs = [mpsum.tile([P, MM_N], f32, tag=f"acc{m}", name=f"acc{m}")
                    for m in range(mc_cnt)]
            dma_idx = 0
            for g in range(3):
                lv = LT[g][:].rearrange("(i t p) m -> i p t m", t=t_pack, p=P)
                for i in range(ni):
                    lt = lpool.tile([P, t_pack * rows], fmm, tag="lt", name="lt")
                    # Rotate rings so consecutive transfers drain concurrently.
                    rings = [nc.sync, nc.scalar, nc.gpsimd][:N_RINGS]
                    eng = rings[dma_idx % len(rings)]
                    dma_idx += 1
                    eng.dma_start(
                        lt[:].rearrange("p (t m) -> p t m", t=t_pack), lv[i])
                    for t in range(t_pack):
                        kt = i * t_pack + t
                        ycol = (kt % J) * c3 * R + (kt // J) * c3 + g * c
                        lhsT = ys[:, ycol:ycol + c]
                        first = g == 0 and i == 0 and t == 0
                        last = g == 2 and i == ni - 1 and t == t_pack - 1
                        for m in range(mc_cnt):
                            m0 = m * MM_N
                            m1 = min(rows, m0 + MM_N)
                            nc.tensor.matmul(
                                accs[m][:c, :m1 - m0],
                                lhsT,
                                lt[:, t * rows + m0:t * rows + m1],
                                start=first, stop=last,
                            )

            outsb = opool.tile([c, rows], f32)
            for m in range(mc_cnt):
                m0 = m * MM_N
                m1 = min(rows, m0 + MM_N)
                nc.scalar.activation(
                    outsb[:, m0:m1], accs[m][:c, :m1 - m0],
                    mybir.ActivationFunctionType.Relu, bias=bs[:, 0:1])
            nc.sync.dma_start(OUT[:], outsb[:])

    nc.compile()
    return nc


def make_in_maps(X, L0, L1, L2, W0, W1, W2, bias, n_cores=N_CORES):
    X = np.ascontiguousarray(np.asarray(X, dtype=np.float32))
    Ls = [np.asarray(L, dtype=np.float32) for L in (L0, L1, L2)]
    Ws = [np.asarray(W, dtype=np.float32) for W in (W0, W1, W2)]
    bias = np.ascontiguousarray(np.asarray(bias, dtype=np.float32))

    n, c = X.shape
    # XT4[32*t + c, i'] = X.T[c, t*(n/R) + i']
    XT4 = np.ascontiguousarray(
        X.T.reshape(c, R, n // R).transpose(1, 0, 2).reshape(P, n // R))
    Wcat = np.concatenate(Ws, axis=1)           # [c, 3c]
    Wcat4 = np.ascontiguousarray(np.tile(Wcat, (R, 1)))  # [128, 3c]

    rows = n // n_cores
    in_maps = []
    for cid in range(n_cores):
        rc = slice(cid * rows, (cid + 1) * rows)
        m = {"XT4": XT4, "Wcat4": Wcat4, "bias": bias}
        for g in range(3):
            m[f"L{g}T"] = np.ascontiguousarray(Ls[g][rc].T)
        in_maps.append(m)
    return in_maps


_NC_CACHE = {}


def _get_nc():
    key = (N, ROWS, T_PACK, LT_BUFS, USE_F32R)
    if key not in _NC_CACHE:
        _NC_CACHE[key] = build_nc()
    return _NC_CACHE[key]


def run(inputs, trace=False, **kwargs):
    nc = _get_nc()
    in_maps = make_in_maps(**inputs)
    res = run_bass_kernel_spmd(nc, in_maps, core_ids=list(range(N_CORES)),
                               trace=trace, **kwargs)
    rows = N // N_CORES
    out = np.empty((N, C), dtype=np.float32)
    for cid in range(N_CORES):
        out[cid * rows:(cid + 1) * rows] = res.results[cid]["out"].T
    return out, res


def kernel(**inputs):
    out, _ = run(inputs, trace=False)
    return out


# revision 17
# speedup vs baseline: 1.2605x; 1.2605x over previous
"""Trainium2 Bass kernel for: out = relu(L0@(X@W0) + L1@(X@W1) + L2@(X@W2) + bias).

Shapes: X [8192, 32], Lk [8192, 8192], Wk [32, 32], bias [32] (all f32).

Strategy (8 NeuronCores, node-dim sharding):
  - Each core owns a 1024-row block of the output: C_c = sum_g Lg[rows_c] @ (X @ Wg).
  - Computed transposed on-chip:  C_c.T = sum_g (X@Wg).T @ Lg[rows_c].T
    so the big Lg data is the *moving* PE operand (streamed through the array)
    and the tiny Y = X@Wg tiles are the stationary weights.
  - PE contracts over the partition dim, so Lg must be laid out with the
    contraction index (Lg's column index) on partitions.  f32 DMA-transpose
    doesn't exist on TRN2, so the transpose is done host-side as part of
    sharding: each core receives LgT_c = Lg[rows_c].T contiguous [8192, 1024].
    That makes every device-side DMA a contiguous line-rate stream - the
    kernel is purely HBM-bandwidth bound (96 MB/core), per the target regime.
  - float32r matmuls: single-pass PE streaming (fp32 needs 2 half-rate
    passes); walrus requires f32r operands to be produced as f32r, so L/X/W
    are declared f32r end-to-end (bit-identical 4-byte storage).
  - Phase A computes Y = X@Wg for all g in one batched pass: X.T is packed
    [128, n/4] (4 row-groups on partitions) and all three W's are multiplied
    concurrently via tile_position row-packing.
  - Per (m-chunk of 512 nodes): one PSUM bank accumulates all 3*64 = 192
    matmuls (graphs x k-tiles); epilogue = ScalarE Relu(acc + bias) -> SBUF
    -> DMA out.  Output per core is C_c.T [32, 1024]; host transposes back.
"""

import numpy as np

import concourse.bacc as bacc
import concourse.mybir as mybir
import concourse.tile as tile
from concourse.bass_utils import run_bass_kernel_spmd

N = 8192
C = 32
N_CORES = 8
ROWS = N // N_CORES  # 1024

P = 128          # SBUF partitions / PE contraction tile
R = 4            # X.T row-group packing factor
MM_N = 512       # max f32 moving free dim (one PSUM bank)
T_PACK = 4       # k-tiles per DMA (DMA size = T_PACK * ROWS * 512B)
LT_BUFS = 7      # L-tile prefetch depth
N_RINGS = 2      # DMA rings to rotate over: sync, scalar (HWDGE), gpsimd (SWDGE)
USE_F32R = True  # float32r matmul (accuracy checked in test.py)


def build_nc(n=N, rows=ROWS, c=C, t_pack=T_PACK, lt_bufs=LT_BUFS,
             use_f32r=USE_F32R, debug=False):
    f32 = mybir.dt.float32
    fmm = mybir.dt.float32r if use_f32r else f32
    kt_total = n // P            # k-tiles per graph
    ni = kt_total // t_pack      # DMA iterations per graph
    mc_cnt = (rows + MM_N - 1) // MM_N
    J = kt_total // R            # inner k-tile groups for phase A packing
    c3 = 3 * c

    nc = bacc.Bacc("TRN2", target_bir_lowering=False, debug=debug)

    XT4 = nc.dram_tensor("XT4", [P, n // R], fmm, kind="ExternalInput")
    Wc4 = nc.dram_tensor("Wcat4", [P, c3], fmm, kind="ExternalInput")
    B = nc.dram_tensor("bias", [c], f32, kind="ExternalInput")
    LT = [nc.dram_tensor(f"L{g}T", [n, rows], fmm, kind="ExternalInput")
          for g in range(3)]
    OUT = nc.dram_tensor("out", [c, rows], f32, kind="ExternalOutput")

    with tile.TileContext(nc) as tc:
        with (
            tc.tile_pool(name="const", bufs=1) as cpool,
            tc.tile_pool(name="ypool", bufs=1) as ypool,
            tc.tile_pool(name="lpool", bufs=lt_bufs) as lpool,
            tc.tile_pool(name="opool", bufs=1) as opool,
            tc.tile_pool(name="apsum", bufs=1, space="PSUM") as apsum,
            tc.tile_pool(name="mpsum", bufs=1, space="PSUM") as mpsum,
        ):
            xt4 = cpool.tile([P, n // R], fmm)
            wc4 = cpool.tile([P, c3], fmm)
            bs = cpool.tile([c, 1], f32)
            nc.sync.dma_start(xt4[:], XT4[:])
            nc.scalar.dma_start(wc4[:], Wc4[:])
            nc.scalar.dma_start(bs[:], B[:][:, None])

            # Phase A: Y_g = X @ Wg for all g at once.
            # ys col layout: (kt % J)*3c*R + (kt // J)*3c + g*c
            ys = ypool.tile([P, kt_total * c3], fmm)
            # Concurrent row-group MMs must land in distinct PSUM banks.
            for j in range(J):
                for s in range(R):
                    pa = apsum.tile([P, c3], f32, tag=f"pa{s}", name=f"pa{s}")
                    nc.tensor.matmul(
                        pa[:],
                        xt4[32 * s:32 * (s + 1), j * P:(j + 1) * P],
                        wc4[32 * s:32 * (s + 1), :],
                        start=True, stop=True,
                        tile_position=(32 * s, 0),
                    )
                    nc.vector.tensor_copy(
                        ys[:, (j * R + s) * c3:(j * R + s + 1) * c3], pa[:])

            # Main: acc_m[:c, :] += Y_g[ktile].T @ LgT[ktile, m-chunk]
            accs = [mpsum.tile([P, MM_N], f32, tag=f"acc{m}", name=f"acc{m}")
                    for m in range(mc_cnt)]
            dma_idx = 0
            for g in range(3):
                lv = LT[g][:].rearrange("(i t p) m -> i p t m", t=t_pack, p=P)
                for i in range(ni):
                    lt = lpool.tile([P, t_pack * rows], fmm, tag="lt", name="lt")
                    # Rotate rings so consecutive transfers drain concurrently.
                    rings = [nc.sync, nc.scalar, nc.gpsimd][:N_RINGS]
                    eng = rings[dma_idx % len(rings)]
                    dma_idx += 1
                    eng.dma_start(
                        lt[:].rearrange("p (t m) -> p t m", t=t_pack), lv[i])
                    for t in range(t_pack):
                        kt = i * t_pack + t
                        ycol = (kt % J) * c3 * R + (kt // J) * c3 + g * c
                        lhsT = ys[:, ycol:ycol + c]
                        first = g == 0 and i == 0 and t == 0
                        last = g == 2 and i == ni - 1 and t == t_pack - 1
                        for m in range(mc_cnt):
                            m0 = m * MM_N
                            m1 = min(rows, m0 + MM_N)
                            nc.tensor.matmul(
                                accs[m][:c, :m1 - m0],
                                lhsT,
                                lt[:, t * rows + m0:t * rows + m1],
                                start=first, stop=last,
                            )

            outsb = opool.tile([c, rows], f32)
            for m in range(mc_cnt):
                m0 = m * MM_N
                m1 = min(rows, m0 + MM_N)
                nc.scalar.activation(
                    outsb[:, m0:m1], accs[m][:c, :m1 - m0],
                    mybir.ActivationFunctionType.Relu, bias=bs[:, 0:1])
            nc.sync.dma_start(OUT[:], outsb[:])

    nc.compile()
    return nc


def make_in_maps(X, L0, L1, L2, W0, W1, W2, bias, n_cores=N_CORES):
    X = np.ascontiguousarray(np.asarray(X, dtype=np.float32))
    Ls = [np.asarray(L, dtype=np.float32) for L in (L0, L1, L2)]
    Ws = [np.asarray(W, dtype=np.float32) for W in (W0, W1, W2)]
    bias = np.ascontiguousarray(np.asarray(bias, dtype=np.float32))

    n, c = X.shape
    # XT4[32*t + c, i'] = X.T[c, t*(n/R) + i']
    XT4 = np.ascontiguousarray(
        X.T.reshape(c, R, n // R).transpose(1, 0, 2).reshape(P, n // R))
    Wcat = np.concatenate(Ws, axis=1)           # [c, 3c]
    Wcat4 = np.ascontiguousarray(np.tile(Wcat, (R, 1)))  # [128, 3c]

    rows = n // n_cores
    in_maps = []
    for cid in range(n_cores):
        rc = slice(cid * rows, (cid + 1) * rows)
        m = {"XT4": XT4, "Wcat4": Wcat4, "bias": bias}
        for g in range(3):
            m[f"L{g}T"] = np.ascontiguousarray(Ls[g][rc].T)
        in_maps.append(m)
    return in_maps


_NC_CACHE = {}


def _get_nc():
    key = (N, ROWS, T_PACK, LT_BUFS, USE_F32R)
    if key not in _NC_CACHE:
        _NC_CACHE[key] = build_nc()
    return _NC_CACHE[key]


def run(inputs, trace=False, **kwargs):
    nc = _get_nc()
    in_maps = make_in_maps(**inputs)
    res = run_bass_kernel_spmd(nc, in_maps, core_ids=list(range(N_CORES)),
                               trace=trace, **kwargs)
    rows = N // N_CORES
    out = np.empty((N, C), dtype=np.float32)
    for cid in range(N_CORES):
        out[cid * rows:(cid + 1) * rows] = res.results[cid]["out"].T
    return out, res


def kernel(**inputs):
    out, _ = run(inputs, trace=False)
    return out
